# revision 16
# baseline (speedup 1.0000x reference)
"""Trainium2 Bass kernel for nn_MultiHeadAttention (B=8, S=2048, D=64, H=8, d_k=8).

Sharding: data-parallel over batch -- core b computes batch b fully
(projections replicated). Outputs: out1 [B,S,D] and attn [B,H,S,S].

Layout note: SBUF access patterns must start at partition 0/32/64/96, so all
per-head data lives in a "spread" layout -- head g occupies 8 partitions at
base 32*(g%4) of an A (heads 0-3) or B (heads 4-7) tile; weight matrices are
rebuilt with matching spread columns/rows (gaps zeroed) so projections and the
output projection are single 128-wide matmuls.

Per-core dataflow:
  - projections: PE-transpose Q/K/V tiles -> QT/KT/VT [64,S]; spread-column
    weight matmuls + ACT bias-copy -> qT/kT spread tiles, v natural tiles.
  - T-path (per head pair, per q-block): scoresT[k,q] via small-K (8) f32r
    matmuls (row-strip packed), exp on ScalarE (PSUM->SBUF) -> E_T, context
    matmul ctxT[d,q] += v[k,d].T @ E_T (contraction k, f32r).
  - N-path: natural scores[q,k] via the same operands with roles swapped,
    exp on ScalarE with accum_out -> unnormalized attn rows in SBUF plus
    softmax denominators; reciprocal + DVE 2x-mode in-place normalize;
    1 MiB DMA per (head, 128 q rows) to HBM.
  - ctx normalize: per-q reciprocal transposed + broadcast via tiny PE
    matmuls, DVE multiply into spread ctxn; out projection via spread WoT
    matmul (A+B accumulate) + residual.
"""

from contextlib import ExitStack

import numpy as np

import concourse.bass as bass
import concourse.bacc as bacc
import concourse.mybir as mybir
import concourse.tile as tile
from concourse.masks import make_identity

F32 = mybir.dt.float32
F32R = mybir.dt.float32r
AF = mybir.ActivationFunctionType

D_MODEL = 64
N_HEADS = 8
D_K = 8
SCALE = 1.0 / np.sqrt(D_K)


def _r(ap, enable=True):
    """bitcast an fp32 AP to float32r for full-rate PE streaming."""
    return ap.bitcast(F32R) if enable else ap


def build_nc(S=2048, f32r_scores=True, f32r_ctx=True):
    """Build the single-core Bass program (SPMD across 8 cores)."""
    assert S % 512 == 0
    NT = S // 128          # number of 128-row tiles (16)
    SB = S // 4            # q-block width (512)
    NSUB = SB // 128       # 128-row sub-blocks per q-block (4)

    nc = bacc.Bacc("TRN2", target_bir_lowering=False)

    Q = nc.declare_dram_parameter("Q", [S, D_MODEL], F32, isOutput=False)
    K = nc.declare_dram_parameter("K", [S, D_MODEL], F32, isOutput=False)
    V = nc.declare_dram_parameter("V", [S, D_MODEL], F32, isOutput=False)
    Wq = nc.declare_dram_parameter("Wq", [D_MODEL, D_MODEL], F32, isOutput=False)
    Wk = nc.declare_dram_parameter("Wk", [D_MODEL, D_MODEL], F32, isOutput=False)
    Wv = nc.declare_dram_parameter("Wv", [D_MODEL, D_MODEL], F32, isOutput=False)
    Wo = nc.declare_dram_parameter("Wo", [D_MODEL, D_MODEL], F32, isOutput=False)
    bq = nc.declare_dram_parameter("bq", [D_MODEL, 1], F32, isOutput=False)
    bk = nc.declare_dram_parameter("bk", [D_MODEL, 1], F32, isOutput=False)
    bv = nc.declare_dram_parameter("bv", [D_MODEL, 1], F32, isOutput=False)
    bo = nc.declare_dram_parameter("bo", [1, D_MODEL], F32, isOutput=False)

    attn_out = nc.declare_dram_parameter("attn", [N_HEADS, S, S], F32, isOutput=True)
    out1 = nc.declare_dram_parameter("out", [S, D_MODEL], F32, isOutput=True)

    with tile.TileContext(nc) as tc, ExitStack() as ctx:
        # ---------------- persistent SBUF pools ----------------
        consts = ctx.enter_context(tc.tile_pool(name="consts", bufs=1))
        ident = consts.tile([128, 128], F32)
        make_identity(nc, ident)
        ones18 = consts.tile([1, 8], F32)
        nc.vector.memset(ones18, 1.0)

        # spread transposed activations: head g at partitions 32*(g%4)..+8
        qTs_A = consts.tile([128, S], F32R)
        qTs_B = consts.tile([128, S], F32R)
        kTs_A = consts.tile([128, S], F32R)
        kTs_B = consts.tile([128, S], F32R)
        v_nat = consts.tile([128, NT, D_MODEL], F32R)   # v natural, tile kt
        Qbo = consts.tile([128, NT, D_MODEL], F32)     # Q + bo residual
        # spread Wo.T rows (head g dims at partitions 32*(g%4), gaps zero)
        WoT_spA = consts.tile([128, D_MODEL], F32)
        WoT_spB = consts.tile([128, D_MODEL], F32)

        def strip(g):
            return 32 * (g % 4)

        def qT_of(g):
            return qTs_A if g < 4 else qTs_B

        def kT_of(g):
            return kTs_A if g < 4 else kTs_B

        # ---------------- projection phase ----------------
        with tc.tile_pool(name="pscr", bufs=2) as ps, \
             tc.tile_pool(name="pscr1", bufs=1) as ps1, \
             tc.tile_pool(name="ppsum", bufs=2, space="PSUM") as pp:
            # raw weights + transposed copies
            WT = {}
            for wname, wdram in (("q", Wq), ("k", Wk), ("v", Wv), ("o", Wo)):
                w_raw = ps.tile([D_MODEL, D_MODEL], F32, tag="wraw", name=f"wraw_{wname}")
                nc.sync.dma_start(out=w_raw, in_=wdram[:, :])
                w_ps = pp.tile([D_MODEL, D_MODEL], F32, tag="pp", name=f"wps_{wname}")
                nc.tensor.transpose(w_ps, w_raw, ident[:D_MODEL, :D_MODEL])
                wT = ps1.tile([D_MODEL, D_MODEL], F32, tag=f"wT_{wname}", name=f"wT_{wname}")
                nc.scalar.copy(wT, w_ps)
                WT[wname] = (w_raw, wT)

            # spread-column projection weights for q, k: col 32u+v = W.T[:, 8u+v]
            Wsp = {}
            for wname in ("q", "k"):
                for half, tname in ((0, "A"), (1, "B")):
                    wsp = ps1.tile([D_MODEL, 128], F32, tag=f"wsp_{wname}{tname}",
                                   name=f"wsp_{wname}{tname}")
                    nc.vector.memset(wsp, 0.0)
                    for u in range(4):
                        g = 4 * half + u
                        nc.vector.tensor_copy(
                            wsp[:, 32 * u:32 * u + 8],
                            WT[wname][1][:, 8 * g:8 * g + 8],
                        )
                    Wsp[(wname, half)] = wsp

            # spread biases bq, bk as [128,1] (partition 32u+v = b[8u+v])
            bsp = {}
            for bname, bdram in (("q", bq), ("k", bk)):
                for half in (0, 1):
                    t = ps1.tile([128, 1], F32, tag=f"bsp_{bname}{half}",
                                 name=f"bsp_{bname}{half}")
                    nc.vector.memset(t, 0.0)
                    for u in range(4):
                        g = 4 * half + u
                        nc.sync.dma_start(out=t[32 * u:32 * u + 8, :],
                                          in_=bdram[8 * g:8 * g + 8, :])
                    bsp[(bname, half)] = t
            bv_sb = ps1.tile([D_MODEL, 1], F32, tag="bv_sb")
            nc.sync.dma_start(out=bv_sb, in_=bv[:, :])

            # spread WoT rows: partitions 32u+v of A = Wo.T[8(4*0+u)+v, :]
            for half, wot_sp in ((0, WoT_spA), (1, WoT_spB)):
                nc.vector.memset(wot_sp, 0.0)
                for u in range(4):
                    g = 4 * half + u
                    # transpose Wo[:, 8g:8g+8] ([64,8]) -> [8,64] psum
                    wsp_ps = pp.tile([8, D_MODEL], F32, tag="pp2",
                                     name=f"wot_ps{half}_{u}")
                    nc.tensor.transpose(
                        wsp_ps,
                        WT["o"][0][:, 8 * g:8 * g + 8],
                        ident[:D_MODEL, :D_MODEL],
                    )
                    nc.scalar.copy(wot_sp[32 * u:32 * u + 8, :], wsp_ps)

            bo_b = ps.tile([128, D_MODEL], F32, tag="bob")
            nc.gpsimd.dma_start(out=bo_b, in_=bo[:, :].to_broadcast((128, D_MODEL)))

            # transposed raw inputs [64, S]
            QT = ps1.tile([D_MODEL, S], F32, tag="QT")
            KT = ps1.tile([D_MODEL, S], F32, tag="KT")
            VT = ps1.tile([D_MODEL, S], F32, tag="VT")
            for xdram, xT in ((Q, QT), (K, KT), (V, VT)):
                for kt in range(NT):
                    x_t = ps.tile([128, D_MODEL], F32, tag="xld")
                    nc.sync.dma_start(out=x_t, in_=xdram[kt * 128:(kt + 1) * 128, :])
                    x_ps = pp.tile([D_MODEL, 128], F32, tag="pp")
                    nc.tensor.transpose(x_ps, x_t, ident)
                    nc.vector.tensor_copy(xT[:, kt * 128:(kt + 1) * 128], x_ps)

            # Q residual (+bo)
            for kt in range(NT):
                q_t = ps.tile([128, D_MODEL], F32, tag="xld")
                nc.sync.dma_start(out=q_t, in_=Q[kt * 128:(kt + 1) * 128, :])
                nc.vector.tensor_add(Qbo[:, kt, :], q_t, bo_b)

            # projections into spread layout
            vT = ps1.tile([D_MODEL, S], F32, tag="vT")
            for qb in range(4):
                sl = slice(qb * SB, (qb + 1) * SB)
                for half, dst_q, dst_k in ((0, qTs_A, kTs_A), (1, qTs_B, kTs_B)):
                    q_ps = pp.tile([128, SB], F32, tag="pj")
                    nc.tensor.matmul(q_ps, lhsT=Wsp[("q", half)], rhs=QT[:, sl],
                                     start=True, stop=True)
                    nc.scalar.add(dst_q[:, sl], q_ps, bsp[("q", half)])
                    k_ps = pp.tile([128, SB], F32, tag="pj")
                    nc.tensor.matmul(k_ps, lhsT=Wsp[("k", half)], rhs=KT[:, sl],
                                     start=True, stop=True)
                    nc.scalar.add(dst_k[:, sl], k_ps, bsp[("k", half)])
                v_ps = pp.tile([D_MODEL, SB], F32, tag="pj")
                nc.tensor.matmul(v_ps, lhsT=WT["v"][1], rhs=VT[:, sl],
                                 start=True, stop=True)
                nc.scalar.add(vT[:, sl], v_ps, bv_sb)
            # v natural tiles
            for kt in range(NT):
                vn_ps = pp.tile([128, D_MODEL], F32, tag="pp")
                nc.tensor.transpose(
                    vn_ps, vT[:, kt * 128:(kt + 1) * 128], ident[:D_MODEL, :D_MODEL]
                )
                nc.scalar.copy(v_nat[:, kt, :], vn_ps)

        # ---------------- main attention loops ----------------
        et_pool = ctx.enter_context(tc.tile_pool(name="et", bufs=4))
        stage = ctx.enter_context(tc.tile_pool(name="stage", bufs=4))
        ctxn_pool = ctx.enter_context(tc.tile_pool(name="ctxn", bufs=2))
        rc = ctx.enter_context(tc.tile_pool(name="rc", bufs=4))
        big_psum = ctx.enter_context(tc.tile_pool(name="bigp", bufs=2, space="PSUM"))
        ctx_psum = ctx.enter_context(tc.tile_pool(name="ctxp", bufs=2, space="PSUM"))

        for qb in range(4):
            qsl = slice(qb * SB, (qb + 1) * SB)
            # spread context (normalized), heads 0-3 in A rows 32g, 4-7 in B
            ctxn_A = ctxn_pool.tile([128, SB], F32, tag="ctxnA")
            ctxn_B = ctxn_pool.tile([128, SB], F32, tag="ctxnB")
            nc.vector.memset(ctxn_A, 0.0)
            nc.vector.memset(ctxn_B, 0.0)
            for p in range(4):
                pair = (2 * p, 2 * p + 1)
                ctx_ps_pair = [
                    ctx_psum.tile([32, SB], F32, tag="ctx", name=f"ctx_g{g}")
                    for g in pair
                ]
                # ---- T path: scoresT + exp + context accumulation ----
                for kt in range(NT):
                    ksl = slice(kt * 128, (kt + 1) * 128)
                    sT = big_psum.tile([128, 2 * SB], F32, tag="big")
                    for j, g in enumerate(pair):
                        st = strip(g)
                        nc.tensor.matmul(
                            sT[:, j * SB:(j + 1) * SB],
                            lhsT=kT_of(g)[st:st + 8, ksl],
                            rhs=qT_of(g)[st:st + 8, qsl],
                            start=True, stop=True, tile_position=(st, 0),
                        )
                    et = et_pool.tile([128, 2 * SB], F32R, tag="et")
                    nc.scalar.activation(out=et, in_=sT, func=AF.Exp, scale=SCALE)
                    for j, g in enumerate(pair):
                        nc.tensor.matmul(
                            ctx_ps_pair[j][0:8, :],
                            lhsT=v_nat[:, kt, 8 * g:8 * g + 8],
                            rhs=et[:, j * SB:(j + 1) * SB],
                            start=(kt == 0), stop=(kt == NT - 1),
                        )
                # ---- N path: natural scores + exp(+accum) + normalize + DMA ----
                for j, g in enumerate(pair):
                    st = strip(g)
                    recip4 = rc.tile([128, NSUB], F32, tag="recip4")
                    for c in range(NSUB):
                        csl = slice(qb * SB + c * 128, qb * SB + c * 128 + 128)
                        stg = stage.tile([128, S], F32, tag="stg")
                        den2 = rc.tile([128, 2], F32, tag="den2")
                        for kb in range(2):
                            sN = big_psum.tile([128, 2 * SB], F32, tag="big")
                            for u in range(2):
                                kbsl = slice((2 * kb + u) * SB, (2 * kb + u + 1) * SB)
                                nc.tensor.matmul(
                                    sN[:, u * SB:(u + 1) * SB],
                                    lhsT=qT_of(g)[st:st + 8, csl],
                                    rhs=kT_of(g)[st:st + 8, kbsl],
                                    start=True, stop=True, tile_position=(st, 0),
                                )
                            nc.scalar.activation(
                                out=stg[:, kb * 2 * SB:(kb + 1) * 2 * SB], in_=sN,
                                func=AF.Exp, scale=SCALE,
                                accum_out=den2[:, kb:kb + 1],
                            )
                        den = rc.tile([128, 1], F32, tag="den1")
                        nc.vector.tensor_add(den, den2[:, 0:1], den2[:, 1:2])
                        nc.vector.reciprocal(recip4[:, c:c + 1], den)
                        nc.vector.tensor_scalar_mul(stg, in0=stg,
                                                    scalar1=recip4[:, c:c + 1])
                        nc.sync.dma_start(out=attn_out[g, csl, :], in_=stg)
                    # ---- ctx normalization for head g ----
                    ctxn_t = ctxn_A if g < 4 else ctxn_B
                    for c in range(NSUB):
                        rT_ps = ctx_psum.tile([1, 128], F32, tag="smalls")
                        nc.tensor.transpose(rT_ps, recip4[:, c:c + 1], ident)
                        rT = rc.tile([1, 128], F32, tag="rT")
                        nc.scalar.copy(rT, rT_ps)
                        bc_ps = ctx_psum.tile([8, 128], F32, tag="smalls")
                        nc.tensor.matmul(bc_ps, lhsT=ones18, rhs=rT,
                                         start=True, stop=True)
                        bc = rc.tile([8, 128], F32, tag="bc")
                        nc.scalar.copy(bc, bc_ps)
                        nc.vector.tensor_mul(
                            ctxn_t[st:st + 8, c * 128:(c + 1) * 128],
                            ctx_ps_pair[j][0:8, c * 128:(c + 1) * 128],
                            bc,
                        )
            # ---- out projection + residual for this q-block ----
            for c in range(NSUB):
                t = qb * NSUB + c
                o_ps = ctx_psum.tile([128, D_MODEL], F32, tag="smalls")
                nc.tensor.matmul(o_ps, lhsT=ctxn_A[:, c * 128:(c + 1) * 128],
                                 rhs=WoT_spA, start=True, stop=False)
                nc.tensor.matmul(o_ps, lhsT=ctxn_B[:, c * 128:(c + 1) * 128],
                                 rhs=WoT_spB, start=False, stop=True)
                o_sb = rc.tile([128, D_MODEL], F32, tag="osb")
                nc.vector.tensor_add(o_sb, o_ps, Qbo[:, t, :])
                nc.sync.dma_start(out=out1[t * 128:(t + 1) * 128, :], in_=o_sb)

    nc.compile()
    return nc


_NC_CACHE = {}


def _get_nc(S=2048):
    key = S
    if key not in _NC_CACHE:
        _NC_CACHE[key] = build_nc(S=S)
    return _NC_CACHE[key]


def _make_in_maps(inputs, B, S):
    common = {
        "Wq": np.ascontiguousarray(inputs["Wq"], dtype=np.float32),
        "Wk": np.ascontiguousarray(inputs["Wk"], dtype=np.float32),
        "Wv": np.ascontiguousarray(inputs["Wv"], dtype=np.float32),
        "Wo": np.ascontiguousarray(inputs["Wo"], dtype=np.float32),
        "bq": np.ascontiguousarray(inputs["bq"], dtype=np.float32).reshape(D_MODEL, 1),
        "bk": np.ascontiguousarray(inputs["bk"], dtype=np.float32).reshape(D_MODEL, 1),
        "bv": np.ascontiguousarray(inputs["bv"], dtype=np.float32).reshape(D_MODEL, 1),
        "bo": np.ascontiguousarray(inputs["bo"], dtype=np.float32).reshape(1, D_MODEL),
    }
    in_maps = []
    for b in range(B):
        m = dict(common)
        m["Q"] = np.ascontiguousarray(inputs["Q"][b], dtype=np.float32)
        m["K"] = np.ascontiguousarray(inputs["K"][b], dtype=np.float32)
        m["V"] = np.ascontiguousarray(inputs["V"][b], dtype=np.float32)
        in_maps.append(m)
    return in_maps


def run(inputs, trace=False, trace_kwargs=None):
    """Run the SPMD kernel on 8 cores; returns (out1, attn, BassKernelResults)."""
    from concourse.bass_utils import run_bass_kernel_spmd

    Qf = np.asarray(inputs["Q"])
    B, S, _ = Qf.shape
    nc = _get_nc(S=S)
    in_maps = _make_in_maps(inputs, B, S)
    kw = {}
    if trace:
        kw["trace"] = True
        if trace_kwargs:
            kw.update(trace_kwargs)
    res = run_bass_kernel_spmd(nc, in_maps, core_ids=list(range(B)), **kw)
    out1 = np.stack([res.results[b]["out"] for b in range(B)], axis=0)
    attn = np.stack([res.results[b]["attn"] for b in range(B)], axis=0)
    return out1, attn, res


def kernel(**inputs):
    out1, attn, _ = run(inputs, trace=False)
    return out1, attn


# revision 18
# speedup vs baseline: 1.0924x; 1.0924x over previous
"""Trainium2 Bass kernel for nn_MultiHeadAttention (B=8, S=2048, D=64, H=8, d_k=8).

Sharding: data-parallel over batch -- core b computes batch b fully
(projections replicated). Outputs: out1 [B,S,D] and attn [B,H,S,S].

Layout note: SBUF access patterns must start at partition 0/32/64/96, so all
per-head data lives in a "spread" layout -- head g occupies 8 partitions at
base 32*(g%4) of an A (heads 0-3) or B (heads 4-7) tile; weight matrices are
rebuilt with matching spread columns/rows (gaps zeroed) so projections and the
output projection are single 128-wide matmuls.

Per-core dataflow:
  - projections: PE-transpose Q/K/V tiles -> QT/KT/VT [64,S]; spread-column
    weight matmuls + ACT bias-copy -> qT/kT spread tiles, v natural tiles.
  - T-path (per head pair, per q-block): scoresT[k,q] via small-K (8) f32r
    matmuls (row-strip packed), exp on ScalarE (PSUM->SBUF) -> E_T, context
    matmul ctxT[d,q] += v[k,d].T @ E_T (contraction k, f32r).
  - N-path: natural scores[q,k] via the same operands with roles swapped,
    exp on ScalarE with accum_out -> unnormalized attn rows in SBUF plus
    softmax denominators; reciprocal + DVE 2x-mode in-place normalize;
    1 MiB DMA per (head, 128 q rows) to HBM.
  - ctx normalize: per-q reciprocal transposed + broadcast via tiny PE
    matmuls, DVE multiply into spread ctxn; out projection via spread WoT
    matmul (A+B accumulate) + residual.
"""

from contextlib import ExitStack

import numpy as np

import concourse.bass as bass
import concourse.bacc as bacc
import concourse.mybir as mybir
import concourse.tile as tile
from concourse.masks import make_identity

F32 = mybir.dt.float32
F32R = mybir.dt.float32r
F16 = mybir.dt.float16
AF = mybir.ActivationFunctionType

D_MODEL = 64
N_HEADS = 8
D_K = 8
SCALE = 1.0 / np.sqrt(D_K)


def _r(ap, enable=True):
    """bitcast an fp32 AP to float32r for full-rate PE streaming."""
    return ap.bitcast(F32R) if enable else ap


def build_nc(S=2048, f32r_scores=True, f32r_ctx=True):
    """Build the single-core Bass program (SPMD across 8 cores)."""
    assert S % 512 == 0
    NT = S // 128          # number of 128-row tiles (16)
    SB = S // 4            # q-block width (512)
    NSUB = SB // 128       # 128-row sub-blocks per q-block (4)

    nc = bacc.Bacc("TRN2", target_bir_lowering=False)

    Q = nc.declare_dram_parameter("Q", [S, D_MODEL], F32, isOutput=False)
    K = nc.declare_dram_parameter("K", [S, D_MODEL], F32, isOutput=False)
    V = nc.declare_dram_parameter("V", [S, D_MODEL], F32, isOutput=False)
    Wq = nc.declare_dram_parameter("Wq", [D_MODEL, D_MODEL], F32, isOutput=False)
    Wk = nc.declare_dram_parameter("Wk", [D_MODEL, D_MODEL], F32, isOutput=False)
    Wv = nc.declare_dram_parameter("Wv", [D_MODEL, D_MODEL], F32, isOutput=False)
    Wo = nc.declare_dram_parameter("Wo", [D_MODEL, D_MODEL], F32, isOutput=False)
    bq = nc.declare_dram_parameter("bq", [D_MODEL, 1], F32, isOutput=False)
    bk = nc.declare_dram_parameter("bk", [D_MODEL, 1], F32, isOutput=False)
    bv = nc.declare_dram_parameter("bv", [D_MODEL, 1], F32, isOutput=False)
    bo = nc.declare_dram_parameter("bo", [1, D_MODEL], F32, isOutput=False)

    attn_out = nc.declare_dram_parameter("attn", [N_HEADS, S, S], F32, isOutput=True)
    out1 = nc.declare_dram_parameter("out", [S, D_MODEL], F32, isOutput=True)

    with tile.TileContext(nc) as tc, ExitStack() as ctx:
        # ---------------- persistent SBUF pools ----------------
        consts = ctx.enter_context(tc.tile_pool(name="consts", bufs=1))
        ident = consts.tile([128, 128], F32)
        make_identity(nc, ident)
        ones18 = consts.tile([1, 8], F16)
        nc.vector.memset(ones18, 1.0)
        warm_sb = consts.tile([128, 512], F16)
        nc.vector.memset(warm_sb, 0.125)

        # spread transposed activations: head g at partitions 32*(g%4)..+8
        qTs_A = consts.tile([128, S], F16)
        qTs_B = consts.tile([128, S], F16)
        kTs_A = consts.tile([128, S], F16)
        kTs_B = consts.tile([128, S], F16)
        v_nat = consts.tile([128, NT, D_MODEL], F32R)   # v natural, tile kt
        Qbo = consts.tile([128, NT, D_MODEL], F32)     # Q + bo residual
        # spread Wo.T rows (head g dims at partitions 32*(g%4), gaps zero)
        WoT_spA = consts.tile([128, D_MODEL], F32)
        WoT_spB = consts.tile([128, D_MODEL], F32)

        def strip(g):
            return 32 * (g % 4)

        def qT_of(g):
            return qTs_A if g < 4 else qTs_B

        def kT_of(g):
            return kTs_A if g < 4 else kTs_B

        # ---------------- projection phase ----------------
        with tc.tile_pool(name="pscr", bufs=2) as ps, \
             tc.tile_pool(name="pscr1", bufs=1) as ps1, \
             tc.tile_pool(name="ppsum", bufs=2, space="PSUM") as pp:
            # raw weights + transposed copies
            WT = {}
            for wname, wdram in (("q", Wq), ("k", Wk), ("v", Wv), ("o", Wo)):
                w_raw = ps.tile([D_MODEL, D_MODEL], F32, tag="wraw", name=f"wraw_{wname}")
                nc.sync.dma_start(out=w_raw, in_=wdram[:, :])
                w_ps = pp.tile([D_MODEL, D_MODEL], F32, tag="pp", name=f"wps_{wname}")
                nc.tensor.transpose(w_ps, w_raw, ident[:D_MODEL, :D_MODEL])
                wT = ps1.tile([D_MODEL, D_MODEL], F32, tag=f"wT_{wname}", name=f"wT_{wname}")
                nc.vector.tensor_copy(wT, w_ps)
                WT[wname] = (w_raw, wT)

            # spread-column projection weights for q, k: col 32u+v = W.T[:, 8u+v]
            Wsp = {}
            for wname in ("q", "k"):
                for half, tname in ((0, "A"), (1, "B")):
                    wsp = ps1.tile([D_MODEL, 128], F32, tag=f"wsp_{wname}{tname}",
                                   name=f"wsp_{wname}{tname}")
                    nc.vector.memset(wsp, 0.0)
                    for u in range(4):
                        g = 4 * half + u
                        nc.vector.tensor_copy(
                            wsp[:, 32 * u:32 * u + 8],
                            WT[wname][1][:, 8 * g:8 * g + 8],
                        )
                    Wsp[(wname, half)] = wsp

            # spread biases bq, bk as [128,1] (partition 32u+v = b[8u+v])
            bsp = {}
            for bname, bdram in (("q", bq), ("k", bk)):
                for half in (0, 1):
                    t = ps1.tile([128, 1], F32, tag=f"bsp_{bname}{half}",
                                 name=f"bsp_{bname}{half}")
                    nc.vector.memset(t, 0.0)
                    for u in range(4):
                        g = 4 * half + u
                        nc.sync.dma_start(out=t[32 * u:32 * u + 8, :],
                                          in_=bdram[8 * g:8 * g + 8, :])
                    bsp[(bname, half)] = t
            bv_sb = ps1.tile([D_MODEL, 1], F32, tag="bv_sb")
            nc.sync.dma_start(out=bv_sb, in_=bv[:, :])

            # spread WoT rows: partitions 32u+v of A = Wo.T[8(4*0+u)+v, :]
            for half, wot_sp in ((0, WoT_spA), (1, WoT_spB)):
                nc.vector.memset(wot_sp, 0.0)
                for u in range(4):
                    g = 4 * half + u
                    # transpose Wo[:, 8g:8g+8] ([64,8]) -> [8,64] psum
                    wsp_ps = pp.tile([8, D_MODEL], F32, tag="pp2",
                                     name=f"wot_ps{half}_{u}")
                    nc.tensor.transpose(
                        wsp_ps,
                        WT["o"][0][:, 8 * g:8 * g + 8],
                        ident[:D_MODEL, :D_MODEL],
                    )
                    nc.vector.tensor_copy(wot_sp[32 * u:32 * u + 8, :], wsp_ps)

            bo_b = ps.tile([128, D_MODEL], F32, tag="bob")
            nc.gpsimd.dma_start(out=bo_b, in_=bo[:, :].to_broadcast((128, D_MODEL)))

            # transposed raw inputs [64, S]
            QT = ps1.tile([D_MODEL, S], F32, tag="QT")
            KT = ps1.tile([D_MODEL, S], F32, tag="KT")
            VT = ps1.tile([D_MODEL, S], F32, tag="VT")
            for xdram, xT in ((Q, QT), (K, KT), (V, VT)):
                for kt in range(NT):
                    x_t = ps.tile([128, D_MODEL], F32, tag="xld")
                    nc.sync.dma_start(out=x_t, in_=xdram[kt * 128:(kt + 1) * 128, :])
                    x_ps = pp.tile([D_MODEL, 128], F32, tag="pp")
                    nc.tensor.transpose(x_ps, x_t, ident)
                    nc.vector.tensor_copy(xT[:, kt * 128:(kt + 1) * 128], x_ps)

            # Q residual (+bo)
            for kt in range(NT):
                q_t = ps.tile([128, D_MODEL], F32, tag="xld")
                nc.sync.dma_start(out=q_t, in_=Q[kt * 128:(kt + 1) * 128, :])
                nc.vector.tensor_add(Qbo[:, kt, :], q_t, bo_b)

            # projections into spread layout
            vT = ps1.tile([D_MODEL, S], F32, tag="vT")
            for qb in range(4):
                sl = slice(qb * SB, (qb + 1) * SB)
                for half, dst_q, dst_k in ((0, qTs_A, kTs_A), (1, qTs_B, kTs_B)):
                    q_ps = pp.tile([128, SB], F32, tag="pj")
                    nc.tensor.matmul(q_ps, lhsT=Wsp[("q", half)], rhs=QT[:, sl],
                                     start=True, stop=True)
                    nc.vector.tensor_scalar(out=dst_q[:, sl], in0=q_ps,
                                            scalar1=bsp[("q", half)], scalar2=None,
                                            op0=mybir.AluOpType.add)
                    k_ps = pp.tile([128, SB], F32, tag="pj")
                    nc.tensor.matmul(k_ps, lhsT=Wsp[("k", half)], rhs=KT[:, sl],
                                     start=True, stop=True)
                    nc.vector.tensor_scalar(out=dst_k[:, sl], in0=k_ps,
                                            scalar1=bsp[("k", half)], scalar2=None,
                                            op0=mybir.AluOpType.add)
                v_ps = pp.tile([D_MODEL, SB], F32, tag="pj")
                nc.tensor.matmul(v_ps, lhsT=WT["v"][1], rhs=VT[:, sl],
                                 start=True, stop=True)
                nc.scalar.add(vT[:, sl], v_ps, bv_sb)
            # v natural tiles
            for kt in range(NT):
                vn_ps = pp.tile([128, D_MODEL], F32, tag="pp")
                nc.tensor.transpose(
                    vn_ps, vT[:, kt * 128:(kt + 1) * 128], ident[:D_MODEL, :D_MODEL]
                )
                nc.vector.tensor_copy(v_nat[:, kt, :], vn_ps)

        # ---------------- main attention loops ----------------
        et_pool = ctx.enter_context(tc.tile_pool(name="et", bufs=4))
        stage = ctx.enter_context(tc.tile_pool(name="stage", bufs=4))
        ctxn_pool = ctx.enter_context(tc.tile_pool(name="ctxn", bufs=2))
        rc = ctx.enter_context(tc.tile_pool(name="rc", bufs=4))
        big_psum = ctx.enter_context(tc.tile_pool(name="bigp", bufs=2, space="PSUM"))
        ctx_psum = ctx.enter_context(tc.tile_pool(name="ctxp", bufs=2, space="PSUM"))

        # PE warm-up: ~8us of back-to-back fp16 matmuls to flip HAM to 8/8
        for w in range(24):
            warm_ps = ctx_psum.tile([8, SB], F32, tag="smalls", name=f"warm{w}")
            nc.tensor.matmul(warm_ps, lhsT=warm_sb[:, 0:8], rhs=warm_sb[:, 0:SB],
                             start=True, stop=True)

        for qb in range(4):
            qsl = slice(qb * SB, (qb + 1) * SB)
            # spread context (normalized), heads 0-3 in A rows 32g, 4-7 in B
            ctxn_A = ctxn_pool.tile([128, SB], F32, tag="ctxnA")
            ctxn_B = ctxn_pool.tile([128, SB], F32, tag="ctxnB")
            nc.vector.memset(ctxn_A, 0.0)
            nc.vector.memset(ctxn_B, 0.0)
            for p in range(4):
                pair = (2 * p, 2 * p + 1)
                ctx_ps_pair = [
                    ctx_psum.tile([32, SB], F32, tag="ctx", name=f"ctx_g{g}")
                    for g in pair
                ]
                # ---- T path: scoresT + exp + context accumulation ----
                for kt in range(NT):
                    ksl = slice(kt * 128, (kt + 1) * 128)
                    sT = big_psum.tile([128, 2 * SB], F32, tag="big")
                    for j, g in enumerate(pair):
                        st = strip(g)
                        nc.tensor.matmul(
                            sT[:, j * SB:(j + 1) * SB],
                            lhsT=kT_of(g)[st:st + 8, ksl],
                            rhs=qT_of(g)[st:st + 8, qsl],
                            start=True, stop=True, tile_position=(st, 0),
                        )
                    et = et_pool.tile([128, 2 * SB], F32R, tag="et")
                    nc.scalar.activation(out=et, in_=sT, func=AF.Exp, scale=SCALE)
                    for j, g in enumerate(pair):
                        nc.tensor.matmul(
                            ctx_ps_pair[j][0:8, :],
                            lhsT=v_nat[:, kt, 8 * g:8 * g + 8],
                            rhs=et[:, j * SB:(j + 1) * SB],
                            start=(kt == 0), stop=(kt == NT - 1),
                        )
                # ---- N path: natural scores + exp(+accum) + normalize + DMA ----
                for j, g in enumerate(pair):
                    st = strip(g)
                    recip4 = rc.tile([128, NSUB], F32, tag="recip4")
                    for c in range(NSUB):
                        csl = slice(qb * SB + c * 128, qb * SB + c * 128 + 128)
                        stg = stage.tile([128, S], F32, tag="stg")
                        den2 = rc.tile([128, 2], F32, tag="den2")
                        for kb in range(2):
                            sN = big_psum.tile([128, 2 * SB], F32, tag="big")
                            for u in range(2):
                                kbsl = slice((2 * kb + u) * SB, (2 * kb + u + 1) * SB)
                                nc.tensor.matmul(
                                    sN[:, u * SB:(u + 1) * SB],
                                    lhsT=qT_of(g)[st:st + 8, csl],
                                    rhs=kT_of(g)[st:st + 8, kbsl],
                                    start=True, stop=True, tile_position=(st, 0),
                                )
                            nc.scalar.activation(
                                out=stg[:, kb * 2 * SB:(kb + 1) * 2 * SB], in_=sN,
                                func=AF.Exp, scale=SCALE,
                                accum_out=den2[:, kb:kb + 1],
                            )
                        den = rc.tile([128, 1], F32, tag="den1")
                        nc.vector.tensor_add(den, den2[:, 0:1], den2[:, 1:2])
                        nc.vector.reciprocal(recip4[:, c:c + 1], den)
                        nc.vector.tensor_scalar_mul(stg, in0=stg,
                                                    scalar1=recip4[:, c:c + 1])
                        nc.sync.dma_start(out=attn_out[g, csl, :], in_=stg)
                    # ---- ctx normalization for head g (batched) ----
                    ctxn_t = ctxn_A if g < 4 else ctxn_B
                    rT_ps = ctx_psum.tile([1, SB], F32, tag="smalls")
                    for c in range(NSUB):
                        nc.tensor.transpose(rT_ps[0:1, c * 128:(c + 1) * 128],
                                            recip4[:, c:c + 1], ident)
                    rT_row = rc.tile([1, SB], F16, tag="rT")
                    nc.vector.tensor_copy(rT_row, rT_ps)
                    bc_ps = ctx_psum.tile([8, SB], F32, tag="smalls")
                    nc.tensor.matmul(bc_ps, lhsT=ones18, rhs=rT_row,
                                     start=True, stop=True)
                    bc = rc.tile([8, SB], F32, tag="bc")
                    nc.vector.tensor_copy(bc, bc_ps)
                    nc.vector.tensor_mul(ctxn_t[st:st + 8, :],
                                         ctx_ps_pair[j][0:8, :], bc)
            # ---- out projection + residual for this q-block ----
            for c in range(NSUB):
                t = qb * NSUB + c
                o_ps = ctx_psum.tile([128, D_MODEL], F32, tag="smalls")
                nc.tensor.matmul(o_ps, lhsT=ctxn_A[:, c * 128:(c + 1) * 128],
                                 rhs=WoT_spA, start=True, stop=False)
                nc.tensor.matmul(o_ps, lhsT=ctxn_B[:, c * 128:(c + 1) * 128],
                                 rhs=WoT_spB, start=False, stop=True)
                o_sb = rc.tile([128, D_MODEL], F32, tag="osb")
                nc.vector.tensor_add(o_sb, o_ps, Qbo[:, t, :])
                nc.sync.dma_start(out=out1[t * 128:(t + 1) * 128, :], in_=o_sb)

    nc.compile()
    return nc


_NC_CACHE = {}


def _get_nc(S=2048):
    key = S
    if key not in _NC_CACHE:
        _NC_CACHE[key] = build_nc(S=S)
    return _NC_CACHE[key]


def _make_in_maps(inputs, B, S):
    common = {
        "Wq": np.ascontiguousarray(inputs["Wq"], dtype=np.float32),
        "Wk": np.ascontiguousarray(inputs["Wk"], dtype=np.float32),
        "Wv": np.ascontiguousarray(inputs["Wv"], dtype=np.float32),
        "Wo": np.ascontiguousarray(inputs["Wo"], dtype=np.float32),
        "bq": np.ascontiguousarray(inputs["bq"], dtype=np.float32).reshape(D_MODEL, 1),
        "bk": np.ascontiguousarray(inputs["bk"], dtype=np.float32).reshape(D_MODEL, 1),
        "bv": np.ascontiguousarray(inputs["bv"], dtype=np.float32).reshape(D_MODEL, 1),
        "bo": np.ascontiguousarray(inputs["bo"], dtype=np.float32).reshape(1, D_MODEL),
    }
    in_maps = []
    for b in range(B):
        m = dict(common)
        m["Q"] = np.ascontiguousarray(inputs["Q"][b], dtype=np.float32)
        m["K"] = np.ascontiguousarray(inputs["K"][b], dtype=np.float32)
        m["V"] = np.ascontiguousarray(inputs["V"][b], dtype=np.float32)
        in_maps.append(m)
    return in_maps


def run(inputs, trace=False, trace_kwargs=None):
    """Run the SPMD kernel on 8 cores; returns (out1, attn, BassKernelResults)."""
    from concourse.bass_utils import run_bass_kernel_spmd

    Qf = np.asarray(inputs["Q"])
    B, S, _ = Qf.shape
    nc = _get_nc(S=S)
    in_maps = _make_in_maps(inputs, B, S)
    kw = {}
    if trace:
        kw["trace"] = True
        if trace_kwargs:
            kw.update(trace_kwargs)
    res = run_bass_kernel_spmd(nc, in_maps, core_ids=list(range(B)), **kw)
    out1 = np.stack([res.results[b]["out"] for b in range(B)], axis=0)
    attn = np.stack([res.results[b]["attn"] for b in range(B)], axis=0)
    return out1, attn, res


def kernel(**inputs):
    out1, attn, _ = run(inputs, trace=False)
    return out1, attn
